# revision 1
# baseline (speedup 1.0000x reference)
"""Trainium2 Bass kernel for nn_EnhancedTelomeraseGNN (GAT x2 + SAGE + pool + MLP).

Strategy (8 NeuronCores, SPMD single NEFF):
- Nodes packed into 128-node chunks; chunks dealt to (core, slot) pairs sorted
  by edge count so per-slot tile counts are balanced across cores (the tile
  schedule is the max over cores and must be SPMD-uniform).
- Dense phases sharded per core; a_s/a_d projections folded into extra W
  columns host-side; results AllGathered into a Shared-HBM bf16 node table
  (row: h bf16[256] | a_s raw-f32[4] | pad, 768B).
- Edge phases: per 128-edge tile, dma_gather of source rows + a second gather
  of per-dst (a_d + r) rows from a core-local table; attention exp weights via
  group-batched DVE/ACT ops; segment-sum via one-hot (bf16, exact) matmul into
  PSUM accumulated per 128-node chunk, two passes (lo/hi table half, int16
  gather index limit).
- Softmax: exp without max-stabilizer (|s| < 9 for this model; identical
  result), 1/den applied after aggregation.
"""
import numpy as np
import ml_dtypes

import concourse.bacc as bacc
import concourse.mybir as mybir
import concourse.tile as tile
from concourse.bass_utils import run_bass_kernel_spmd
from concourse.library_config import mlp
from concourse.masks import make_identity

N, E, F_IN, HID, HEADS, NT, B = 50000, 400000, 32, 64, 4, 20, 64
IN1 = F_IN + HID          # 96
F = HEADS * HID           # 256
P = 128
M = 8                     # cores
GT = 8                    # tiles per gather group
NI = GT * P               # idxs per gather
ELEM = 384                # bf16 table row elems (h 256 | a_s f32-as-8bf16 | pad)
ADEL = 64                 # a_d table row elems f32 (a_d+r 4 | pad), 256B
DEXT = 264                # dense out cols: h 256 | a_s 4 | a_d+r 4

f32 = mybir.dt.float32
f32r = mybir.dt.float32r
bf16 = mybir.dt.bfloat16
i16 = mybir.dt.int16
i32 = mybir.dt.int32
AF = mybir.ActivationFunctionType
OP = mybir.AluOpType
BF = ml_dtypes.bfloat16


def _pack_gather_idx(idx_flat):
    v = idx_flat.reshape(-1, 16).T.astype(np.int16)
    return np.tile(v, (8, 1)).copy()


def build_schedule(src, dst):
    K = (N + P - 1) // P
    order = np.argsort(dst, kind="stable")
    dsts = dst[order]
    srcs = src[order]
    edges_per_chunk = np.bincount(dsts // P, minlength=K)
    chunk_start = np.concatenate([[0], np.cumsum(edges_per_chunk)])

    # deal chunks to (core, slot): sorted by edge count desc, slot s gets the
    # ranked chunks [M*s, M*s+M) -> per-slot max over cores ~ per-slot mean
    S = (K + M - 1) // M
    rank = np.argsort(-edges_per_chunk, kind="stable")
    chunk_core = np.full(K, -1, dtype=np.int64)
    chunk_slot = np.full(K, -1, dtype=np.int64)
    for r, k in enumerate(rank):
        chunk_slot[k] = r // M
        chunk_core[k] = r % M
    node_chunk = np.arange(N) // P
    node_row = (chunk_core[node_chunk] * S + chunk_slot[node_chunk]) * P + (np.arange(N) % P)
    NROWS = M * S * P
    HALF = NROWS // 2

    slot_chunk = np.full((M, S), -1, dtype=np.int64)
    for k in range(K):
        slot_chunk[chunk_core[k], chunk_slot[k]] = k

    tiles = np.zeros((2, M, S), dtype=np.int64)
    per_cs = {}
    for c in range(M):
        for s in range(S):
            k = slot_chunk[c, s]
            if k < 0:
                continue
            e0, e1 = chunk_start[k], chunk_start[k + 1]
            rows = node_row[srcs[e0:e1]]
            lo = rows < HALF
            for pss, msk in enumerate([lo, ~lo]):
                idx = np.nonzero(msk)[0] + e0
                tiles[pss, c, s] = (len(idx) + P - 1) // P
                per_cs[(pss, c, s)] = idx
    tiles_sched = tiles.max(axis=1)
    tiles_sched[0] = np.maximum(tiles_sched[0], 1)  # pass-L copy must init acc
    for pss in range(2):
        tiles_sched[pss, S - 1] += (-int(tiles_sched[pss].sum())) % GT
    T_tot = int(tiles_sched.sum())

    gidx = np.zeros((M, T_tot * P), dtype=np.int32)
    dloc = np.full((M, T_tot * P), 999.0, dtype=np.float32)
    adix = np.zeros((M, T_tot * P), dtype=np.int32)
    slot_of_tile = np.zeros(T_tot, dtype=np.int64)
    pass_of_tile = np.zeros(T_tot, dtype=np.int64)
    t_base = 0
    for pss in range(2):
        for s in range(S):
            ntl = int(tiles_sched[pss, s])
            slot_of_tile[t_base:t_base + ntl] = s
            pass_of_tile[t_base:t_base + ntl] = pss
            for c in range(M):
                idx = per_cs.get((pss, c, s), np.zeros(0, dtype=np.int64))
                n = len(idx)
                b0 = t_base * P
                gidx[c, b0:b0 + n] = node_row[srcs[idx]] - (HALF if pss else 0)
                dloc[c, b0:b0 + n] = (dsts[idx] % P).astype(np.float32)
                adix[c, b0:b0 + n] = s * P + dsts[idx] % P
            t_base += ntl

    return dict(order=order, dsts=dsts, srcs=srcs, S=S, K=K,
                NROWS=NROWS, HALF=HALF, node_row=node_row,
                tiles_sched=tiles_sched, T_tot=T_tot, gidx=gidx, dloc=dloc,
                adix=adix, slot_of_tile=slot_of_tile,
                pass_of_tile=pass_of_tile, per_cs=per_cs)


def build_nc(S, tiles_sched, T_tot, phases=99):
    NROWS = M * S * P
    HALF = NROWS // 2
    SR = S * P

    nc = bacc.Bacc(num_devices=M, num_swdge_queues=4)
    groups = [list(range(M))]

    xT = nc.dram_tensor("xT", [F_IN, SR], f32, kind="ExternalInput")
    ohntT = nc.dram_tensor("ohntT", [NT, SR], f32, kind="ExternalInput")
    idx_main = nc.dram_tensor("idx_main", [128, T_tot * 8], i16, kind="ExternalInput")
    idx_ad = nc.dram_tensor("idx_ad", [128, T_tot * 8], i16, kind="ExternalInput")
    dst_all = nc.dram_tensor("dst_all", [128, T_tot], bf16, kind="ExternalInput")
    ea_all = nc.dram_tensor("ea_all", [128, T_tot], f32, kind="ExternalInput")
    inv_deg = nc.dram_tensor("inv_deg", [128, S], f32, kind="ExternalInput")
    bo_in = nc.dram_tensor("bo_in", [128, S * P], bf16, kind="ExternalInput")
    W1_in = nc.dram_tensor("W1_in", [IN1 + 1, DEXT], f32, kind="ExternalInput")
    W2_in = nc.dram_tensor("W2_in", [HID + 1, DEXT], f32, kind="ExternalInput")
    emb_in = nc.dram_tensor("emb_in", [NT, P], f32, kind="ExternalInput")
    qr_c = nc.dram_tensor("qr_c", [128, 2 * HEADS], f32, kind="ExternalInput")
    b1_rep = nc.dram_tensor("b1_rep", [128, HID], f32, kind="ExternalInput")
    b2_rep = nc.dram_tensor("b2_rep", [128, HID], f32, kind="ExternalInput")
    sageb_rep = nc.dram_tensor("sageb_rep", [128, HID], f32, kind="ExternalInput")
    Wlr_in = nc.dram_tensor("Wlr_in", [2 * HID, HID], f32, kind="ExternalInput")
    lin1W_in = nc.dram_tensor("lin1W_in", [HID, P], f32, kind="ExternalInput")
    lin2W_in = nc.dram_tensor("lin2W_in", [HID, P], f32, kind="ExternalInput")
    lin1b = nc.dram_tensor("lin1b", [HID, 1], f32, kind="ExternalInput")
    lin2b = nc.dram_tensor("lin2b", [HID, 1], f32, kind="ExternalInput")
    headW_in = nc.dram_tensor("headW_in", [HID, 4], f32, kind="ExternalInput")
    headb_rep = nc.dram_tensor("headb_rep", [B, 3], f32, kind="ExternalInput")
    invcnt_rep = nc.dram_tensor("invcnt_rep", [128, 1], f32, kind="ExternalInput")

    out3 = nc.dram_tensor("out3", [B, 3], f32, kind="ExternalOutput")

    table_shard = nc.dram_tensor("table_shard", [SR, ELEM], bf16)
    adtab = nc.dram_tensor("adtab", [SR, ADEL], f32)
    table_sh1 = nc.dram_tensor("table_sh1", [NROWS, ELEM], bf16, addr_space="Shared")
    table_sh2 = nc.dram_tensor("table_sh2", [NROWS, ELEM], bf16, addr_space="Shared")
    t3_shard = nc.dram_tensor("t3_shard", [SR, P], bf16)
    table_sh3 = nc.dram_tensor("table_sh3", [NROWS, P], bf16, addr_space="Shared")
    ar_in = nc.dram_tensor("ar_in", [128, HID], f32)
    ar_out = nc.dram_tensor("ar_out", [128, HID], f32, addr_space="Shared")

    sched = []
    for pss in range(2):
        for s in range(S):
            ntl = int(tiles_sched[pss][s])
            for k in range(ntl):
                sched.append((pss, s, k == 0, k == ntl - 1))
    assert len(sched) == T_tot

    with tile.TileContext(nc) as tc:
        with tc.tile_pool(name="const", bufs=1) as cpool, \
             tc.tile_pool(name="persist", bufs=1) as pers, \
             tc.tile_pool(name="work", bufs=2) as work, \
             tc.tile_pool(name="gath", bufs=2) as gpool, \
             tc.tile_pool(name="ps_out", bufs=2, space="PSUM") as ps_out, \
             tc.tile_pool(name="ps_misc", bufs=4, space="PSUM") as ps_misc, \
             tc.tile_pool(name="ps_hold", bufs=1, space="PSUM") as ps_hold:
            nc.gpsimd.load_library(mlp)

            ident = cpool.tile([P, P], f32)
            make_identity(nc, ident[:])
            iota_i = cpool.tile([P, P], i32)
            nc.gpsimd.iota(iota_i[:], pattern=[[1, P]], base=0, channel_multiplier=0)
            iota_b = cpool.tile([P, P], bf16)
            nc.vector.tensor_copy(iota_b[:], iota_i[:])

            def load_const(drt, shape, dtype=f32):
                t = cpool.tile(shape, dtype, tag=f"c_{drt.name}")
                nc.sync.dma_start(t[:], drt[:])
                return t

            def load_const_r(drt, shape):
                t = work.tile(shape, f32, tag="cload")
                nc.sync.dma_start(t[:], drt[:])
                r = cpool.tile(shape, f32r, tag=f"cr_{drt.name}")
                nc.vector.tensor_copy(r[:], t[:])
                return r

            W1c = load_const_r(W1_in, [IN1 + 1, DEXT])
            W2c = load_const_r(W2_in, [HID + 1, DEXT])
            embc = load_const_r(emb_in, [NT, P])
            Wlrc = load_const_r(Wlr_in, [2 * HID, HID])
            lin1Wc = load_const_r(lin1W_in, [HID, P])
            lin2Wc = load_const_r(lin2W_in, [HID, P])
            headWc = load_const_r(headW_in, [HID, 4])
            qrc = load_const(qr_c, [128, 2 * HEADS])  # [q1 | q2]
            b_rep = {1: load_const(b1_rep, [128, HID]), 2: load_const(b2_rep, [128, HID])}
            sagebc = load_const(sageb_rep, [128, HID])
            lin1bc = load_const(lin1b, [HID, 1])
            lin2bc = load_const(lin2b, [HID, 1])
            headbc = load_const(headb_rep, [B, 3])
            invcntc = load_const(invcnt_rep, [128, 1])

            idxm_sb = pers.tile([128, T_tot * 8], i16)
            nc.sync.dma_start(idxm_sb[:], idx_main[:])
            idxa_sb = pers.tile([128, T_tot * 8], i16)
            nc.sync.dma_start(idxa_sb[:], idx_ad[:])
            dst_sb = pers.tile([128, T_tot], bf16)
            nc.sync.dma_start(dst_sb[:], dst_all[:])
            ea_sb = pers.tile([128, T_tot], f32)
            nc.sync.dma_start(ea_sb[:], ea_all[:])
            invdeg_sb = pers.tile([128, S], f32)
            nc.sync.dma_start(invdeg_sb[:], inv_deg[:])
            bo_b = pers.tile([128, S * P], bf16)
            nc.sync.dma_start(bo_b[:], bo_in[:])

            acc = pers.tile([128, S * (F + HEADS)], f32)
            h1T_sb = pers.tile([HID + 1, SR], f32r)
            nc.vector.tensor_scalar(out=h1T_sb[HID:HID + 1, :], in0=bo_b[0:1, 0:SR],
                                    scalar1=0.0, scalar2=1.0, op0=OP.mult, op1=OP.add)
            h2_sb = pers.tile([128, S * HID], f32)
            SQ = (S + 3) // 4
            tmin_q = pers.tile([128, SQ * HID], f32)
            e_q = pers.tile([128, SQ * HID], f32)

            # ================= dense phase =================
            def dense_phase(layer):
                Wc = W1c if layer == 1 else W2c
                for s in range(S):
                    if layer == 1:
                        xf = work.tile([F_IN, P], f32, tag="xf")
                        nc.sync.dma_start(xf[:], xT[:, s * P:(s + 1) * P])
                        of = work.tile([NT, P], f32, tag="of")
                        nc.sync.dma_start(of[:], ohntT[:, s * P:(s + 1) * P])
                        orr = work.tile([NT, P], f32r, tag="orr")
                        nc.vector.tensor_copy(orr[:], of[:])
                        h0T = work.tile([IN1 + 1, P], f32r, tag="h0T")
                        nc.vector.tensor_copy(h0T[HID:IN1, :], xf[:])
                        nc.vector.tensor_scalar(out=h0T[IN1:IN1 + 1, :], in0=xf[0:1, :],
                                                scalar1=0.0, scalar2=1.0,
                                                op0=OP.mult, op1=OP.add)
                        eps = ps_misc.tile([P, P], f32, space="PSUM", tag="mps")
                        nc.tensor.matmul(eps[:], embc[:], orr[:], start=True, stop=True)
                        nc.vector.tensor_copy(h0T[:HID, :], eps[0:HID, :])
                        lhsT = h0T[:]
                    else:
                        lhsT = h1T_sb[:, s * P:(s + 1) * P]
                    hps = ps_misc.tile([P, DEXT], f32, space="PSUM", tag="mps")
                    nc.tensor.matmul(hps[:], lhsT, Wc[:], start=True, stop=True)
                    tt = work.tile([P, ELEM], bf16, tag="ttile")
                    nc.scalar.activation(tt[:, 0:F], hps[:, 0:F], AF.Copy)
                    nc.vector.tensor_copy(tt[:, F:F + 8].bitcast(f32), hps[:, F:F + HEADS])
                    adt = work.tile([P, ADEL], f32, tag="adt")
                    nc.vector.tensor_copy(adt[:, 0:HEADS], hps[:, F + HEADS:DEXT])
                    nc.sync.dma_start(table_shard[s * P:(s + 1) * P, :], tt[:])
                    nc.sync.dma_start(adtab[s * P:(s + 1) * P, :], adt[:])

            # ================= edge phase (GAT) =================
            def edge_phase(layer, table_sh):
                qoff = 0 if layer == 1 else HEADS
                po = gt = oh_g = None
                for t, (pss, s, first, last) in enumerate(sched):
                    g, ti = t // GT, t % GT
                    if ti == 0:
                        gt = gpool.tile([128, GT, ELEM], bf16, tag="gt")
                        src_ap = table_sh[0:HALF, :] if pss == 0 else table_sh[HALF:NROWS, :]
                        nc.gpsimd.dma_gather(
                            gt[:], src_ap, idxm_sb[:, g * (NI // 16):(g + 1) * (NI // 16)],
                            NI, NI, ELEM, queue_num=(2 * g) % 4,
                            single_packet=False)
                        adg = gpool.tile([128, GT, ADEL], f32, tag="adg")
                        nc.gpsimd.dma_gather(
                            adg[:], adtab[:], idxa_sb[:, g * (NI // 16):(g + 1) * (NI // 16)],
                            NI, NI, ADEL, queue_num=(2 * g + 1) % 4,
                            single_packet=False)
                        # s = a_s + (a_d + r) + ea*q ; lrelu(0.2) ; ex = exp(s)
                        sae = work.tile([128, GT, HEADS], f32, tag="sae")
                        nc.vector.tensor_tensor(
                            out=sae[:],
                            in0=ea_sb[:, g * GT:(g + 1) * GT].to_broadcast([128, GT, HEADS]),
                            in1=qrc[:, qoff:qoff + HEADS][:, None, :]
                                .to_broadcast([128, GT, HEADS]),
                            op=OP.mult)
                        nc.vector.tensor_tensor(
                            out=sae[:], in0=sae[:],
                            in1=gt[:, :, F:F + 8].bitcast(f32), op=OP.add)
                        nc.vector.tensor_tensor(
                            out=sae[:], in0=sae[:], in1=adg[:, :, 0:HEADS], op=OP.add)
                        lr = work.tile([128, GT, HEADS], f32, tag="lr")
                        nc.vector.tensor_scalar(
                            out=lr[:], in0=sae[:], scalar1=0.2, scalar2=None, op0=OP.mult)
                        nc.vector.tensor_tensor(out=sae[:], in0=sae[:], in1=lr[:], op=OP.max)
                        # ex -> gt[:, :, 256:260] (a_s bytes, already consumed)
                        nc.scalar.activation(gt[:, :, F:F + HEADS], sae[:], AF.Exp)
                        # msg = h * ex, in place
                        nc.vector.tensor_tensor(
                            out=gt[:, :, 0:F].rearrange("p t (h c) -> p t h c", h=HEADS),
                            in0=gt[:, :, 0:F].rearrange("p t (h c) -> p t h c", h=HEADS),
                            in1=gt[:, :, F:F + HEADS].to_broadcast([128, GT, HEADS, HID]),
                            op=OP.mult)
                        oh_g = work.tile([128, GT, P], bf16, tag="ohg")
                        nc.vector.tensor_tensor(
                            out=oh_g[:],
                            in0=iota_b[:][:, None, :].to_broadcast([128, GT, P]),
                            in1=dst_sb[:, g * GT:(g + 1) * GT].to_broadcast([128, GT, P]),
                            op=OP.is_equal)
                    if first:
                        po = ps_out.tile([P, F + HEADS], f32, space="PSUM", tag="po")
                    nc.tensor.matmul(po[:], oh_g[:, ti, :], gt[:, ti, 0:F + HEADS],
                                     start=first, stop=last)
                    if last:
                        a_sl = acc[:, s * (F + HEADS):(s + 1) * (F + HEADS)]
                        if pss == 0:
                            nc.scalar.activation(a_sl, po[:], AF.Copy)
                        else:
                            nc.vector.tensor_tensor(out=a_sl, in0=a_sl, in1=po[:], op=OP.add)

            # ================= GAT finalize (slot-quarter batches) =============
            def gat_finalize(layer):
                CW = F + HEADS
                SQ = (S + 3) // 4
                for q0 in range(0, S, SQ):
                    nq = min(SQ, S - q0)
                    blk = acc[:, q0 * CW:(q0 + nq) * CW].rearrange("p (s w) -> p s w", w=CW)
                    inv = work.tile([128, SQ, HEADS], f32, tag="inv")
                    nc.vector.tensor_scalar(out=inv[:, 0:nq, :], in0=blk[:, :, F:F + HEADS],
                                            scalar1=1e-16, scalar2=None, op0=OP.add)
                    nc.vector.reciprocal(inv[:, 0:nq, :], inv[:, 0:nq, :])
                    hblk = blk[:, :, 0:F]
                    nc.vector.tensor_tensor(
                        out=hblk.rearrange("p s (h c) -> p s h c", h=HEADS),
                        in0=hblk.rearrange("p s (h c) -> p s h c", h=HEADS),
                        in1=inv[:, 0:nq, :].to_broadcast([128, nq, HEADS, HID]),
                        op=OP.mult)
                    ob = h2_sb[:, q0 * HID:(q0 + nq) * HID]
                    o = ob.rearrange("p (s c) -> p s c", c=HID)
                    nc.vector.tensor_reduce(
                        out=o, in_=hblk.rearrange("p s (h c) -> p s c h", h=HEADS),
                        axis=mybir.AxisListType.X, op=OP.add)
                    nc.vector.tensor_scalar(out=ob, in0=ob, scalar1=1.0 / HEADS,
                                            scalar2=None, op0=OP.mult)
                    nc.vector.tensor_tensor(
                        out=o, in0=o,
                        in1=b_rep[layer][:][:, None, :].to_broadcast([128, nq, HID]),
                        op=OP.add)
                    tm = tmin_q[:, 0:nq * HID]
                    nc.vector.tensor_scalar(out=tm, in0=ob, scalar1=0.0,
                                            scalar2=None, op0=OP.min)
                    ee = e_q[:, 0:nq * HID]
                    nc.scalar.activation(ee, tm, AF.Exp)
                    nc.vector.tensor_tensor(out=ob, in0=ob, in1=tm, op=OP.subtract)
                    nc.vector.tensor_tensor(out=ob, in0=ob, in1=ee, op=OP.add)
                    nc.vector.tensor_scalar(out=ob, in0=ob, scalar1=1.0,
                                            scalar2=None, op0=OP.subtract)
                if layer == 1:
                    for s in range(S):
                        tps = ps_misc.tile([HID, P], f32, space="PSUM", tag="mps")
                        nc.tensor.transpose(tps[:], h2_sb[:, s * HID:(s + 1) * HID], ident[:])
                        nc.vector.tensor_copy(h1T_sb[0:HID, s * P:(s + 1) * P], tps[:])
                else:
                    h2b = work.tile([128, S * HID], bf16, tag="h2b")
                    nc.vector.tensor_copy(h2b[:], h2_sb[:])
                    nc.sync.dma_start(
                        t3_shard[:].rearrange("(s j) e -> j s e", j=P)[:, :, 0:HID],
                        h2b[:].rearrange("p (s c) -> p s c", c=HID))

            # ================= SAGE edge phase =================
            def sage_edge():
                po = gt3 = oh_g = None
                for t, (pss, s, first, last) in enumerate(sched):
                    g, ti = t // GT, t % GT
                    if ti == 0:
                        gt3 = gpool.tile([128, GT, P], bf16, tag="gt3")
                        src_ap = (table_sh3[0:HALF, :] if pss == 0
                                  else table_sh3[HALF:NROWS, :])
                        nc.gpsimd.dma_gather(
                            gt3[:], src_ap, idxm_sb[:, g * (NI // 16):(g + 1) * (NI // 16)],
                            NI, NI, P, queue_num=g % 4,
                            single_packet=False)
                        oh_g = work.tile([128, GT, P], bf16, tag="ohg")
                        nc.vector.tensor_tensor(
                            out=oh_g[:],
                            in0=iota_b[:][:, None, :].to_broadcast([128, GT, P]),
                            in1=dst_sb[:, g * GT:(g + 1) * GT].to_broadcast([128, GT, P]),
                            op=OP.is_equal)
                    if first:
                        po = ps_out.tile([P, HID], f32, space="PSUM", tag="po")
                    nc.tensor.matmul(po[:], oh_g[:, ti, :], gt3[:, ti, 0:HID],
                                     start=first, stop=last)
                    if last:
                        a_sl = acc[:, s * HID:(s + 1) * HID]
                        if pss == 0:
                            nc.scalar.activation(a_sl, po[:], AF.Copy)
                        else:
                            nc.vector.tensor_tensor(out=a_sl, in0=a_sl, in1=po[:], op=OP.add)

            # ================= SAGE finalize + pooling + heads ================
            def sage_finalize():
                gps = ps_hold.tile([P, HID], f32, space="PSUM", tag="poolps")
                for s in range(S):
                    cat = work.tile([P, 2 * HID], f32, tag="cat")
                    nc.vector.tensor_scalar(
                        out=cat[:, 0:HID], in0=acc[:, s * HID:(s + 1) * HID],
                        scalar1=invdeg_sb[:, s:s + 1], scalar2=None, op0=OP.mult)
                    nc.vector.tensor_copy(cat[:, HID:], h2_sb[:, s * HID:(s + 1) * HID])
                    tps = ps_misc.tile([P, P], f32, space="PSUM", tag="mps")
                    nc.tensor.transpose(tps[:], cat[:], ident[:])
                    catT = work.tile([2 * HID, P], f32r, tag="catT")
                    nc.vector.tensor_copy(catT[:], tps[:])
                    h3ps = ps_misc.tile([P, HID], f32, space="PSUM", tag="mps")
                    nc.tensor.matmul(h3ps[:], catT[:], Wlrc[:], start=True, stop=True)
                    o = work.tile([P, HID], f32, tag="o3")
                    nc.vector.tensor_tensor(out=o[:], in0=h3ps[:], in1=sagebc[:], op=OP.add)
                    tmin = work.tile([P, HID], f32, tag="tmin3")
                    nc.vector.tensor_scalar(out=tmin[:], in0=o[:], scalar1=0.0,
                                            scalar2=None, op0=OP.min)
                    e = work.tile([P, HID], f32, tag="e3")
                    nc.scalar.activation(e[:], tmin[:], AF.Exp)
                    nc.vector.tensor_tensor(out=o[:], in0=o[:], in1=tmin[:], op=OP.subtract)
                    nc.vector.tensor_tensor(out=o[:], in0=o[:], in1=e[:], op=OP.add)
                    h3b = work.tile([P, HID], bf16, tag="h3b")
                    nc.vector.tensor_scalar(out=h3b[:], in0=o[:], scalar1=1.0,
                                            scalar2=None, op0=OP.subtract)
                    nc.tensor.matmul(gps[:], bo_b[:, s * P:(s + 1) * P], h3b[:],
                                     start=(s == 0), stop=(s == S - 1))
                pool_sb = work.tile([P, HID], f32, tag="poolsb")
                nc.vector.tensor_copy(pool_sb[:], gps[:])
                nc.sync.dma_start(ar_in[:], pool_sb[:])
                nc.gpsimd.collective_compute(
                    "AllReduce", OP.add, replica_groups=groups,
                    ins=[ar_in[:]], outs=[ar_out[:]])
                gdiv = work.tile([P, P], f32, tag="gdiv")
                nc.gpsimd.memset(gdiv[:], 0.0)
                gs = work.tile([P, HID], f32, tag="gs")
                nc.sync.dma_start(gs[:], ar_out[:])
                nc.vector.tensor_scalar(out=gdiv[:, 0:HID], in0=gs[:],
                                        scalar1=invcntc[:, 0:1], scalar2=None, op0=OP.mult)
                gtp = ps_misc.tile([P, P], f32, space="PSUM", tag="mps")
                nc.tensor.transpose(gtp[:], gdiv[:], ident[:])
                gT = work.tile([HID, B], f32r, tag="gT")
                nc.vector.tensor_copy(gT[:], gtp[0:HID, 0:B])
                l1ps = ps_misc.tile([P, B], f32, space="PSUM", tag="mps")
                nc.tensor.matmul(l1ps[:], lin1Wc[:], gT[:], start=True, stop=True)
                g1 = work.tile([HID, B], f32, tag="g1")
                nc.vector.tensor_scalar(out=g1[:], in0=l1ps[0:HID, :], scalar1=lin1bc[:, 0:1],
                                        scalar2=None, op0=OP.add)
                tmin = work.tile([HID, B], f32, tag="tminl")
                nc.vector.tensor_scalar(out=tmin[:], in0=g1[:], scalar1=0.0,
                                        scalar2=None, op0=OP.min)
                e = work.tile([HID, B], f32, tag="el")
                nc.scalar.activation(e[:], tmin[:], AF.Exp)
                nc.vector.tensor_tensor(out=g1[:], in0=g1[:], in1=tmin[:], op=OP.subtract)
                nc.vector.tensor_tensor(out=g1[:], in0=g1[:], in1=e[:], op=OP.add)
                g1r = work.tile([HID, B], f32r, tag="g1r")
                nc.vector.tensor_scalar(out=g1r[:], in0=g1[:], scalar1=1.0,
                                        scalar2=None, op0=OP.subtract)
                l2ps = ps_misc.tile([P, B], f32, space="PSUM", tag="mps")
                nc.tensor.matmul(l2ps[:], lin2Wc[:], g1r[:], start=True, stop=True)
                g2 = work.tile([HID, P], f32r, tag="g2")
                nc.vector.tensor_scalar(out=g2[:], in0=iota_b[0:HID, :],
                                        scalar1=0.0, scalar2=None, op0=OP.mult)
                nc.vector.tensor_scalar(out=g2[:, 0:B], in0=l2ps[0:HID, :], scalar1=lin2bc[:, 0:1],
                                        scalar2=None, op0=OP.add)
                hps3 = ps_misc.tile([P, 4], f32, space="PSUM", tag="mps")
                nc.tensor.matmul(hps3[:], g2[:], headWc[:], start=True, stop=True)
                ot = work.tile([B, 3], f32, tag="ot")
                nc.vector.tensor_tensor(out=ot[:], in0=hps3[0:B, 0:3], in1=headbc[:], op=OP.add)
                nc.sync.dma_start(out3[:], ot[:])

            def dump_debug():
                nc.sync.dma_start(out3[:], headbc[:])

            dense_phase(1)
            if phases >= 2:
                nc.gpsimd.collective_compute(
                    "AllGather", OP.bypass, replica_groups=groups,
                    ins=[table_shard[:]], outs=[table_sh1[:]])
            if phases >= 3:
                edge_phase(1, table_sh1)
            if phases >= 4:
                gat_finalize(1)
            if phases >= 5:
                dense_phase(2)
            if phases >= 6:
                nc.gpsimd.collective_compute(
                    "AllGather", OP.bypass, replica_groups=groups,
                    ins=[table_shard[:]], outs=[table_sh2[:]])
                edge_phase(2, table_sh2)
            if phases >= 7:
                gat_finalize(2)
            if phases >= 8:
                nc.gpsimd.collective_compute(
                    "AllGather", OP.bypass, replica_groups=groups,
                    ins=[t3_shard[:]], outs=[table_sh3[:]])
                sage_edge()
            if phases >= 9:
                sage_finalize()
            else:
                dump_debug()

    nc.compile()
    return nc


_CACHE = {}


def _install_ntff_shim(so_path="/opt/axon/libaxon_pjrt.so"):
    import sys, types, ctypes, contextlib, os
    if "antenv.axon_hooks" in sys.modules:
        return
    hook = [None]
    if os.path.exists(so_path):
        try:
            lib = ctypes.CDLL(so_path)
            if hasattr(lib, "axon_start_nrt_profile"):
                lib.axon_start_nrt_profile.argtypes = [
                    ctypes.POINTER(ctypes.c_int64), ctypes.c_size_t]
                lib.axon_start_nrt_profile.restype = ctypes.c_int64
                lib.axon_stop_nrt_profile.argtypes = [ctypes.c_char_p]
                lib.axon_stop_nrt_profile.restype = ctypes.c_int64

                @contextlib.contextmanager
                def _hook(output_dir, device_ids):
                    import jax
                    jax.devices()
                    if device_ids:
                        ids = (ctypes.c_int64 * len(device_ids))(*device_ids)
                        rc = lib.axon_start_nrt_profile(ids, len(device_ids))
                    else:
                        rc = lib.axon_start_nrt_profile(None, 0)
                    if rc != 0:
                        raise RuntimeError(f"axon_start_nrt_profile rc={rc}")
                    try:
                        yield
                    finally:
                        n = lib.axon_stop_nrt_profile(str(output_dir).encode())
                        print(f"ntff profile: {n} file(s) -> {output_dir}")

                hook[0] = _hook
        except OSError:
            pass
    mod = types.ModuleType("antenv.axon_hooks")
    mod.get_axon_ntff_profile_hook = lambda: hook[0]
    mod.set_axon_ntff_profile_hook = lambda h: hook.__setitem__(0, h)
    sys.modules["antenv.axon_hooks"] = mod


_install_ntff_shim()


def _prepare(inputs):
    inp = {k: np.ascontiguousarray(np.asarray(v)) for k, v in inputs.items()}
    src = inp["edge_index"][0].astype(np.int64)
    dst = inp["edge_index"][1].astype(np.int64)
    sch = build_schedule(src, dst)
    S, T_tot = sch["S"], sch["T_tot"]
    SR = S * P

    def qr(We, att_e):
        q = ((inp["edge_W"][0] @ We).reshape(HEADS, HID) * att_e).sum(-1)
        r = ((inp["edge_b"] @ We).reshape(HEADS, HID) * att_e).sum(-1)
        return q.astype(np.float32), r.astype(np.float32)

    q1, r1 = qr(inp["We1"], inp["att_e1"])
    q2, r2 = qr(inp["We2"], inp["att_e2"])
    qr_c = np.tile(np.concatenate([q1, q2])[None, :], (128, 1)).astype(np.float32)

    def wext(W_rows, att_s, att_d, rvec, perm_emb_first):
        Wp = W_rows
        if perm_emb_first:
            Wp = np.concatenate([W_rows[F_IN:], W_rows[:F_IN]])
        scol = np.einsum("fhc,hc->fh", Wp.reshape(-1, HEADS, HID),
                         np.asarray(att_s, np.float32))
        dcol = np.einsum("fhc,hc->fh", Wp.reshape(-1, HEADS, HID),
                         np.asarray(att_d, np.float32))
        top = np.concatenate([Wp, scol, dcol], axis=1)
        ones_row = np.concatenate([np.zeros(F + HEADS, np.float32), rvec])
        return np.concatenate([top, ones_row[None, :]]).astype(np.float32)

    W1e = wext(inp["W1"], inp["att_src1"], inp["att_dst1"], r1, True)
    W2e = wext(inp["W2"], inp["att_src2"], inp["att_dst2"], r2, False)

    ea_sorted = inp["edge_attr"][sch["order"]].astype(np.float32)
    deg = np.bincount(sch["dsts"], minlength=N).astype(np.float32)
    inv_deg_full = (1.0 / np.maximum(deg, 1.0)).astype(np.float32)
    batch = inp["batch"].astype(np.int64)
    cnt = np.bincount(batch, minlength=B).astype(np.float32)
    invcnt = np.zeros((128, 1), dtype=np.float32)
    invcnt[:B, 0] = 1.0 / np.maximum(cnt, 1.0)

    common = dict(
        W1_in=W1e, W2_in=W2e,
        emb_in=np.pad(inp["node_emb"], ((0, 0), (0, P - HID))).astype(np.float32),
        qr_c=qr_c,
        b1_rep=np.tile(inp["b1"][None, :], (128, 1)).astype(np.float32),
        b2_rep=np.tile(inp["b2"][None, :], (128, 1)).astype(np.float32),
        sageb_rep=np.tile(inp["sage_b"][None, :], (128, 1)).astype(np.float32),
        Wlr_in=np.concatenate([inp["sage_Wl"], inp["sage_Wr"]], axis=0).astype(np.float32),
        lin1W_in=np.pad(inp["lin1_W"], ((0, 0), (0, P - HID))).astype(np.float32),
        lin2W_in=np.pad(inp["lin2_W"], ((0, 0), (0, P - HID))).astype(np.float32),
        lin1b=inp["lin1_b"].reshape(HID, 1).astype(np.float32),
        lin2b=inp["lin2_b"].reshape(HID, 1).astype(np.float32),
        headW_in=np.pad(np.concatenate(
            [inp["tel_W"], inp["comp_W"], inp["pch_W"]], axis=1),
            ((0, 0), (0, 1))).astype(np.float32),
        headb_rep=np.tile(np.array(
            [inp["tel_b"][0], inp["comp_b"][0], inp["pch_b"][0]],
            dtype=np.float32)[None, :], (B, 1)),
        invcnt_rep=invcnt,
    )

    in_maps = []
    x = inp["x"].astype(np.float32)
    ntype = inp["node_type"].astype(np.int64)
    node_row = sch["node_row"]
    for c in range(M):
        xT_c = np.zeros((F_IN, SR), dtype=np.float32)
        ohnt = np.zeros((NT, SR), dtype=np.float32)
        msk = (node_row >= c * SR) & (node_row < (c + 1) * SR)
        nn = np.nonzero(msk)[0]
        loc = node_row[nn] - c * SR
        xT_c[:, loc] = x[nn].T
        ohnt[ntype[nn], loc] = 1.0
        invdeg_c = np.zeros((128, S), dtype=np.float32)
        invdeg_c[loc % P, loc // P] = inv_deg_full[nn]
        bo = np.zeros((128, S * P), dtype=BF)
        bo[loc % P, (loc // P) * P + batch[nn]] = 1.0

        eai = np.full(T_tot * P, -1, dtype=np.int64)
        t_base = 0
        for pss in range(2):
            for s in range(S):
                ntl = int(sch["tiles_sched"][pss, s])
                idx = sch["per_cs"].get((pss, c, s))
                if idx is not None and len(idx):
                    eai[t_base * P: t_base * P + len(idx)] = idx
                t_base += ntl
        eav = np.where(eai >= 0, ea_sorted[np.maximum(eai, 0)], 0.0).astype(np.float32)
        ea_c = eav.reshape(T_tot, P).T.copy()
        dst_c = sch["dloc"][c].reshape(T_tot, P).T.astype(BF)

        in_maps.append(dict(
            xT=xT_c, ohntT=ohnt,
            idx_main=_pack_gather_idx(sch["gidx"][c]),
            idx_ad=_pack_gather_idx(sch["adix"][c]),
            dst_all=dst_c, ea_all=ea_c,
            inv_deg=invdeg_c, bo_in=bo, **common))
    return sch, in_maps


def _get_compiled(sch):
    import os
    phases = int(os.environ.get("K_PHASES", "99"))
    key = (sch["S"], tuple(map(tuple, sch["tiles_sched"])), sch["T_tot"], phases)
    if key not in _CACHE:
        _CACHE[key] = build_nc(sch["S"], sch["tiles_sched"], sch["T_tot"], phases)
    return _CACHE[key]


def kernel(**inputs):
    sch, in_maps = _prepare(inputs)
    nc = _get_compiled(sch)
    res = run_bass_kernel_spmd(nc, in_maps, core_ids=list(range(M)))
    o = res.results[0]["out3"].astype(np.float32)
    return (o[:, 0:1].copy(), o[:, 1:2].copy(), o[:, 2:3].copy())


def run_traced(inputs, **kw):
    sch, in_maps = _prepare(inputs)
    nc = _get_compiled(sch)
    res = run_bass_kernel_spmd(nc, in_maps, core_ids=list(range(M)), **kw)
    o = res.results[0]["out3"].astype(np.float32)
    return (o[:, 0:1].copy(), o[:, 1:2].copy(), o[:, 2:3].copy()), res

